# revision 1
# baseline (speedup 1.0000x reference)
"""Trainium2 Bass kernel for the pairwise concordance-index loss.

reference:
    loss = sum_{i<j, f_i=f_j=1} relu((p_i-p_j)(t_i-t_j)) / 100 / n_pairs

Math:
  M[i,j] = f_i f_j (p_i-p_j)(t_i-t_j) = A^T B, rank 4:
      A = [f*u, f, f*p, f*t],  B = [f, f*u, -f*t, -f*p],  u = p*t
  (flags fold in because relu(f_i f_j x) = f_i f_j relu(x) for 0/1 flags)
  sum relu(M) = 0.5*(sum M + sum |M|); sum M has an O(B) closed form done
  on the host in fp64; sum |M| is the O(B^2) part done on device.

Device decomposition (8 cores, identical program, data-sharded):
  64 row-blocks of 128 rows; core k owns blocks 8k..8k+7 as two gangs of
  4. Each block processes cyclic column-offsets e=0..32 (cols 128a+128e
  mod 8192): e=1..31 at weight 1; e=0 / e=32 at weight 0.5 via
  0.5-pre-scaled slab appendices (host-side), so all device sums have
  uniform weight.

Device structure per gang (4 row-blocks in lockstep):
  K=4 bf16 matmuls generate M. The 4 blocks' matmuls are packed into
  disjoint 32-row PE groups via tile_position (rows 0/32/64/96) and run
  CONCURRENTLY (~3x PE throughput; K=4 matmuls never warm the HAM clock,
  so concurrency is the only lever). Each "quad" (4 concurrent N<=512
  matmuls) fills the 4 banks of one [128, 4, 512] PSUM tile, which is
  consumed by ONE abs-row-sum job on either the DVE
  (tensor_reduce(apply_absolute_value, axis=XY)) or the ScalarE
  (activation(Abs, accum_out)), alternating to balance both engines.
"""

import numpy as np

B = 8192
P = 128
NCORE = 8
ABLK = 8            # row-blocks per core (2 gangs of 4)
BMAIN = 5120        # main slab: 128*(7 + 33)
BCOLS = BMAIN + 2 * ABLK * P
E0_OFF = BMAIN                 # 0.5*cols[128a ..+128) at E0_OFF+128a
E32_OFF = BMAIN + ABLK * P     # 0.5*cols[128a+4096 ..+128) at E32_OFF+128a

# per gang: Q1..Q7 (N=512 quads), Q8 (N=384 quad), Q9 (e32|e0 two N=128 quads)
NJOBS = 36          # 18 two-bank tiles per gang x 2 gangs

_cache = {}


def _build():
    """Build + compile the Bass module (once per process)."""
    import concourse.bacc as bacc
    import concourse.tile as tile
    import concourse.mybir as mybir

    f32 = mybir.dt.float32
    bf16 = mybir.dt.bfloat16
    nc = bacc.Bacc("TRN2", target_bir_lowering=False, debug=False, num_devices=NCORE)

    a_dram = nc.dram_tensor("a_rows", [P, 2 * P], bf16, kind="ExternalInput")
    b_dram = nc.dram_tensor("b_cols", [4, BCOLS], bf16, kind="ExternalInput")
    acc_dram = nc.dram_tensor("acc", [P, NJOBS], f32, kind="ExternalOutput")

    with tile.TileContext(nc) as tc:
        with (
            tc.tile_pool(name="inp", bufs=1) as inp_pool,
            tc.tile_pool(name="accp", bufs=1) as acc_pool,
            tc.tile_pool(name="ps", bufs=4, space="PSUM") as ps,
        ):
            a_sb = inp_pool.tile([P, 2 * P], bf16)
            nc.sync.dma_start(a_sb[:, :], a_dram.ap()[:, :])
            # replicate the 4 B-factor rows into all four 32-row groups;
            # chunk columns so the first-needed cols land first
            b_sb = inp_pool.tile([P, BCOLS], bf16)
            CUT = 2688
            for q in range(4):
                nc.sync.dma_start(
                    b_sb[32 * q:32 * q + 4, 0:CUT], b_dram.ap()[:, 0:CUT]
                )
            for q in range(4):
                nc.sync.dma_start(
                    b_sb[32 * q:32 * q + 4, CUT:BCOLS], b_dram.ap()[:, CUT:BCOLS]
                )

            acc_sb = acc_pool.tile([P, NJOBS], f32)

            job = 0
            for g in range(2):          # gangs: row-blocks 4g..4g+3
                def pair(poff, coff_of_a, n, engines, g=g):
                    """One quad split across two 2-bank tiles + their reduces.
                    engines: (engine for tile A [q0,q1], engine for tile B)."""
                    nonlocal job
                    tiles = (
                        ps.tile([P, 2, 512], f32, tag="q", name=f"qa{job}"),
                        ps.tile([P, 2, 512], f32, tag="q", name=f"qb{job}"),
                    )
                    for q in range(4):
                        coff = coff_of_a(4 * g + q)
                        nc.tensor.matmul(
                            tiles[q // 2][:, q % 2, poff:poff + n],
                            a_sb[32 * q:32 * q + 4, P * g:P * g + P],
                            b_sb[32 * q:32 * q + 4, coff:coff + n],
                            start=True,
                            stop=True,
                            tile_position=(32 * q, 0),
                        )
                    return tiles

                def reduce_tile(red, use_dve):
                    nonlocal job
                    if use_dve:
                        nc.vector.tensor_reduce(
                            acc_sb[:, job:job + 1], red,
                            axis=mybir.AxisListType.XY, op=mybir.AluOpType.add,
                            apply_absolute_value=True,
                        )
                    else:
                        nc.scalar.activation(
                            red, red,
                            mybir.ActivationFunctionType.Abs,
                            accum_out=acc_sb[:, job:job + 1],
                        )
                    job += 1

                # Q1..Q7: e=1..28 (N=512). tileA->ACT, tileB->DVE, except the
                # last quad of gang 1 sends both to ACT for balance.
                for s in range(7):
                    both_act = (g == 1 and s == 6)
                    tA, tB = pair(0, lambda a, s=s: P * a + 128 * (1 + 4 * s), 512,
                                  None)
                    reduce_tile(tA[:, :, :], use_dve=False)
                    reduce_tile(tB[:, :, :], use_dve=not both_act)
                # Q8: e29..31 (N=384) -> DVE
                tA, tB = pair(0, lambda a: P * a + 128 * 29, 384, None)
                reduce_tile(tA[:, :, 0:384], use_dve=True)
                reduce_tile(tB[:, :, 0:384], use_dve=True)
                # Q9: e32 then e0 (both N=128, pre-scaled) -> DVE
                t9 = None
                tA, tB = pair(0, lambda a: E32_OFF + P * a, 128, None)
                for q in range(4):
                    coff = E0_OFF + P * (4 * g + q)
                    nc.tensor.matmul(
                        (tA, tB)[q // 2][:, q % 2, 128:256],
                        a_sb[32 * q:32 * q + 4, P * g:P * g + P],
                        b_sb[32 * q:32 * q + 4, coff:coff + 128],
                        start=True,
                        stop=True,
                        tile_position=(32 * q, 0),
                    )
                reduce_tile(tA[:, :, 0:256], use_dve=True)
                reduce_tile(tB[:, :, 0:256], use_dve=True)

            assert job == NJOBS, job
            nc.sync.dma_start(acc_dram.ap()[:, :], acc_sb[:, :])

    nc.compile()
    return nc


def _get_nc():
    if "nc" not in _cache:
        _cache["nc"] = _build()
    return _cache["nc"]


def _make_in_maps(p, t, f, u):
    import ml_dtypes

    A = np.ascontiguousarray(
        np.stack([f * u, f, f * p, f * t]).astype(ml_dtypes.bfloat16)
    )
    Bm = np.ascontiguousarray(
        np.stack([f, f * u, -f * t, -f * p]).astype(ml_dtypes.bfloat16)
    )
    Bh = Bm * np.asarray(0.5, dtype=ml_dtypes.bfloat16)  # exact halving

    in_maps = []
    for k in range(NCORE):
        # a_rows layout: row 32q+r = factor r of row-block 4g+q, cols 128g..+128
        a_rows = np.zeros((P, 2 * P), dtype=ml_dtypes.bfloat16)
        for g in range(2):
            for q in range(4):
                a = 4 * g + q
                rows = slice(1024 * k + P * a, 1024 * k + P * a + P)
                a_rows[32 * q:32 * q + 4, P * g:P * g + P] = A[:, rows]

        b_cols = np.empty((4, BCOLS), dtype=ml_dtypes.bfloat16)
        cols = (1024 * k + np.arange(BMAIN)) % B
        b_cols[:, 0:BMAIN] = Bm[:, cols]
        e0_cols = (1024 * k + np.arange(ABLK * P)) % B
        b_cols[:, E0_OFF:E0_OFF + ABLK * P] = Bh[:, e0_cols]
        e32_cols = (1024 * k + 4096 + np.arange(ABLK * P)) % B
        b_cols[:, E32_OFF:E32_OFF + ABLK * P] = Bh[:, e32_cols]
        in_maps.append(
            {"a_rows": a_rows, "b_cols": np.ascontiguousarray(b_cols)}
        )
    return in_maps, A, Bm


def kernel(pred, gt, gt_fracTime, gt_ifMOF):
    from concourse import bass_utils

    pred = np.asarray(pred)
    gt = np.asarray(gt)
    ift = int(np.asarray(gt_fracTime))
    imf = int(np.asarray(gt_ifMOF))

    p = pred.astype(np.float32)
    t = gt[:, ift].astype(np.float32)
    f = (gt[:, imf] == 1).astype(np.float32)
    u = (p * t).astype(np.float32)

    in_maps, A, Bm = _make_in_maps(p, t, f, u)
    nc = _get_nc()
    res = bass_utils.run_bass_kernel_spmd(nc, in_maps, core_ids=list(range(NCORE)))

    # T = sum_{i<j} ff |M| (all device accumulator columns are weight 1)
    T = 0.0
    for r in res.results:
        T += r["acc"].astype(np.float64).sum()

    # host closed form in fp64 over the same bf16 values the device used:
    # sum_{i<j} M = (sum_{i,j} M - sum_diag M) / 2
    A64 = A.astype(np.float64)
    B64 = Bm.astype(np.float64)
    S_all = (A64.sum(axis=1) * B64.sum(axis=1)).sum()
    D_diag = (A64 * B64).sum()
    S_half = (S_all - D_diag) / 2.0

    f64 = f.astype(np.float64)
    S_f = f64.sum()
    n_pairs = (S_f * S_f - S_f) / 2.0

    loss = 0.5 * (S_half + T) / 100.0 / n_pairs
    return np.asarray(np.float32(loss))



# revision 11
# speedup vs baseline: 1.3869x; 1.3869x over previous
"""Trainium2 Bass kernel for the pairwise concordance-index loss.

reference:
    loss = sum_{i<j, f_i=f_j=1} relu((p_i-p_j)(t_i-t_j)) / 100 / n_pairs

Math:
  Only flagged (f=1) entries contribute, so the host first COMPACTS the
  arrays to the n1 flagged entries (padded with zeros to NB*128), which
  shrinks the pairwise matrix from B^2 to ~(0.7B)^2.
  M[i,j] = (p_i-p_j)(t_i-t_j) = A^T B, rank 4:
      A = [u, 1, p, t],  B = [1, u, -t, -p],  u = p*t   (zeros in padding)
  sum relu(M) = 0.5*(sum M + sum |M|); sum M has an O(n) closed form done
  on the host in fp64; sum |M| is the O(n^2) part done on device.

Device decomposition (8 cores, identical program, data-sharded):
  NB row-blocks of 128 rows; core k owns blocks NBC*k..NBC*k+NBC-1 as a
  quad gang (4 blocks) + duo gang (NBC-4 blocks). Each block processes
  cyclic column-offsets e=0..NB/2 (cols 128a+128e mod NB*128): e=1..NB/2-1
  at weight 1; e=0 / e=NB/2 at weight 0.5 via 0.5-pre-scaled slab
  appendices (host-side), so all device sums have uniform weight.

Device structure per gang (blocks in lockstep):
  K=4 bf16 matmuls generate M, packed into disjoint 32-row PE groups via
  tile_position and run concurrently. Each slot fills (part of) a
  [128, 2, 512] PSUM tile consumed by ONE fused abs-row-sum job on either
  the DVE (tensor_reduce(apply_absolute_value)) or the ScalarE
  (activation(Abs, accum_out)), greedily балансed across both engines.
  Input DMAs are split across the sync and scalar HWDGE queues and
  chunked so the duo gang can start before all replicas have landed.
"""

import numpy as np

B = 8192
P = 128
NCORE = 8

_cache = {}


def _plan(n1):
    """Compile-time plan derived from the flagged count."""
    nb = max(1, -(-n1 // P))        # 128-row blocks needed
    nb = -(-nb // NCORE) * NCORE    # multiple of 8 (even)
    nbc = nb // NCORE               # blocks per core
    eh = nb // 2                    # antipodal offset (weight 1/2)
    mainw = P * (nbc - 1 + eh - 1)  # shared slab for e=1..eh-1
    tailw = nbc * 256               # per-block [e0/2 | e_h/2] appendix
    gangs = [4] * (nbc // 4) + ([nbc % 4] if nbc % 4 else [])
    w = P * (eh - 1)                # main cols per block
    nfull, rem = w // 512, w % 512
    return dict(nb=nb, nbc=nbc, eh=eh, mainw=mainw, tailw=tailw,
                bcols=mainw + tailw, gangs=gangs, nfull=nfull, rem=rem)


def _slots(plan):
    """Interleaved (gang, kind, n) slot list; duo first for early start."""
    gangs = plan["gangs"]
    per_gang = []
    for g, sz in enumerate(gangs):
        sl = [(g, "main", s) for s in range(plan["nfull"])]
        if plan["rem"]:
            sl.append((g, "rem", plan["nfull"]))
        sl.append((g, "tail", 0))
        per_gang.append(sl)
    order = []
    ng = len(per_gang)
    nmax = max(len(s) for s in per_gang)
    for i in range(nmax):
        for g in reversed(range(ng)):  # duo (last gang) first
            if i < len(per_gang[g]):
                order.append(per_gang[g][i])
    return order


def _build(plan):
    """Build + compile the Bass module (once per plan)."""
    import concourse.bacc as bacc
    import concourse.tile as tile
    import concourse.mybir as mybir

    f32 = mybir.dt.float32
    bf16 = mybir.dt.bfloat16
    nc = bacc.Bacc("TRN2", target_bir_lowering=False, debug=False,
                   num_devices=NCORE)

    gangs = plan["gangs"]
    bcols = plan["bcols"]
    mainw = plan["mainw"]
    rem = plan["rem"]
    nfull = plan["nfull"]
    awidth = P * len(gangs)

    slots = _slots(plan)
    # jobs: (slot index, FD) in emission order, for engine balancing
    njobs = 0
    for g, kind, s in slots:
        njobs += 2 if gangs[g] > 2 else 1

    a_dram = nc.dram_tensor("a_rows", [P, awidth], bf16, kind="ExternalInput")
    b_dram = nc.dram_tensor("b_cols", [4, bcols], bf16, kind="ExternalInput")
    acc_dram = nc.dram_tensor("acc", [P, njobs], f32, kind="ExternalOutput")

    with tile.TileContext(nc) as tc:
        with (
            tc.tile_pool(name="inp", bufs=1) as inp_pool,
            tc.tile_pool(name="accp", bufs=1) as acc_pool,
            tc.tile_pool(name="ps", bufs=4, space="PSUM") as ps,
        ):
            a_sb = inp_pool.tile([P, awidth], bf16)
            b_sb = inp_pool.tile([P, bcols], bf16)
            # scalar queue: lhs factors (small, needed by every gang), then
            # the Abs table load + ACT jobs follow on the same engine.
            import os
            a_eng = nc.sync if os.environ.get("A_ON_SYNC") else nc.scalar
            a_eng.dma_start(a_sb[:, :], a_dram.ap()[:, :])
            # sync queue: the 4 rhs replicas, first-needed columns first.
            CUT = min(2560, bcols)
            for q in range(4):
                nc.sync.dma_start(b_sb[32 * q:32 * q + 4, 0:CUT],
                                  b_dram.ap()[:, 0:CUT])
            if CUT < bcols:
                for q in range(4):
                    nc.sync.dma_start(b_sb[32 * q:32 * q + 4, CUT:bcols],
                                      b_dram.ap()[:, CUT:bcols])

            acc_sb = acc_pool.tile([P, njobs], f32)

            # greedy DVE/ACT balance using measured per-job cost models
            vt, at = [600.0], [2000.0]

            def reduce_tile(red, job):
                cv = 125.0 + red.free_size() * 1.042
                ca = 330.0 + red.free_size() * 0.834
                if vt[0] + cv <= at[0] + ca:
                    vt[0] += cv
                    nc.vector.tensor_reduce(
                        acc_sb[:, job:job + 1], red,
                        axis=mybir.AxisListType.XY, op=mybir.AluOpType.add,
                        apply_absolute_value=True,
                    )
                else:
                    at[0] += ca
                    nc.scalar.activation(
                        red, red,
                        mybir.ActivationFunctionType.Abs,
                        accum_out=acc_sb[:, job:job + 1],
                    )

            job = 0
            for g, kind, s in slots:
                sz = gangs[g]
                off = sum(gangs[:g])          # first block of this gang
                acol = P * g                  # lhs column range for gang
                ntiles = 2 if sz > 2 else 1
                tiles = [ps.tile([P, 2, 512], f32, tag="q",
                                 name=f"t{job}_{i}") for i in range(ntiles)]
                # Concurrent matmuls (distinct PE row-groups) must target
                # DISTINCT PSUM banks; only same-row-group matmuls may share
                # a bank (they serialize).  Tails therefore run as two
                # sequential waves of N=128.
                if kind == "tail":
                    for wave in range(2):
                        for q in range(sz):
                            coff = mainw + 256 * (off + q) + 128 * wave
                            if ntiles == 2:
                                out = tiles[q // 2][:, q % 2,
                                                    128 * wave:128 * wave + 128]
                            else:
                                out = tiles[0][:, q,
                                               128 * wave:128 * wave + 128]
                            nc.tensor.matmul(
                                out,
                                a_sb[32 * q:32 * q + 4, acol:acol + P],
                                b_sb[32 * q:32 * q + 4, coff:coff + 128],
                                start=True,
                                stop=True,
                                tile_position=(32 * q, 0),
                            )
                else:
                    for q in range(sz):
                        if kind == "main":
                            coff, n = P * (off + q) + 512 * s, 512
                        else:
                            coff, n = P * (off + q) + 512 * s, rem
                        if ntiles == 2:
                            out = tiles[q // 2][:, q % 2, 0:n]
                        else:
                            out = tiles[0][:, q, 0:n]
                        nc.tensor.matmul(
                            out,
                            a_sb[32 * q:32 * q + 4, acol:acol + P],
                            b_sb[32 * q:32 * q + 4, coff:coff + n],
                            start=True,
                            stop=True,
                            tile_position=(32 * q, 0),
                        )
                # one reduce job per tile
                for i, t in enumerate(tiles):
                    nmm = min(sz - 2 * i, 2)  # matmuls landed in this tile
                    if kind == "main":
                        red = t[:, :, :] if nmm == 2 else t[:, 0:1, :]
                    elif kind == "rem":
                        red = t[:, 0:nmm, 0:rem]
                    else:  # tail: nmm banks, cols 0:256 each
                        red = t[:, 0:nmm, 0:256]
                    reduce_tile(red, job)
                    job += 1

            assert job == njobs, (job, njobs)
            nc.sync.dma_start(acc_dram.ap()[:, :], acc_sb[:, :])

    nc.compile()
    return nc


def _get_nc(plan):
    key = ("nc", plan["nb"])
    if key not in _cache:
        _cache[key] = _build(plan)
    return _cache[key]


def _prepare(pred, gt, ift, imf):
    """Compact + pad + build per-core input maps."""
    import ml_dtypes

    p_full = np.asarray(pred).astype(np.float32)
    gt = np.asarray(gt).astype(np.float32)
    t_full = gt[:, ift]
    f_full = gt[:, imf] == 1
    idx = np.flatnonzero(f_full)
    n1 = len(idx)

    plan = _plan(n1)
    npad = plan["nb"] * P
    p = np.zeros(npad, np.float32)
    t = np.zeros(npad, np.float32)
    w = np.zeros(npad, np.float32)
    p[:n1] = p_full[idx]
    t[:n1] = t_full[idx]
    w[:n1] = 1.0
    u = p * t

    # compaction makes the flags trivial: real entries are all flagged,
    # padded entries are exactly zero in every factor.
    A = np.ascontiguousarray(
        np.stack([u, w, p, t]).astype(ml_dtypes.bfloat16)
    )
    Bm = np.ascontiguousarray(
        np.stack([w, u, -t, -p]).astype(ml_dtypes.bfloat16)
    )
    Bh = Bm * np.asarray(0.5, dtype=ml_dtypes.bfloat16)  # exact halving

    nbc, eh, mainw = plan["nbc"], plan["eh"], plan["mainw"]
    gangs = plan["gangs"]
    awidth = P * len(gangs)
    in_maps = []
    for k in range(NCORE):
        a_rows = np.zeros((P, awidth), dtype=ml_dtypes.bfloat16)
        for g, sz in enumerate(gangs):
            off = sum(gangs[:g])
            for q in range(sz):
                blk = nbc * k + off + q
                a_rows[32 * q:32 * q + 4, P * g:P * g + P] = \
                    A[:, P * blk:P * blk + P]

        b_colsk = np.empty((4, plan["bcols"]), dtype=ml_dtypes.bfloat16)
        cols = (P * (nbc * k + 1) + np.arange(mainw)) % npad
        b_colsk[:, 0:mainw] = Bm[:, cols]
        for a in range(nbc):
            c0 = (P * (nbc * k + a) + np.arange(P)) % npad
            ch = (P * (nbc * k + a + eh) + np.arange(P)) % npad
            base = mainw + 256 * a
            b_colsk[:, base:base + P] = Bh[:, c0]
            b_colsk[:, base + P:base + 256] = Bh[:, ch]
        in_maps.append(
            {"a_rows": a_rows, "b_cols": np.ascontiguousarray(b_colsk)}
        )
    return in_maps, A, Bm, n1, plan


def kernel(pred, gt, gt_fracTime, gt_ifMOF):
    from concourse import bass_utils

    ift = int(np.asarray(gt_fracTime))
    imf = int(np.asarray(gt_ifMOF))

    in_maps, A, Bm, n1, plan = _prepare(pred, gt, ift, imf)
    nc = _get_nc(plan)
    res = bass_utils.run_bass_kernel_spmd(nc, in_maps,
                                          core_ids=list(range(NCORE)))

    # T = sum_{i<j} |M| (all device accumulator columns are weight 1)
    T = 0.0
    for r in res.results:
        T += r["acc"].astype(np.float64).sum()

    # host closed form in fp64 over the same bf16 values the device used:
    # sum_{i<j} M = (sum_{i,j} M - sum_diag M) / 2
    A64 = A.astype(np.float64)
    B64 = Bm.astype(np.float64)
    S_all = (A64.sum(axis=1) * B64.sum(axis=1)).sum()
    D_diag = (A64 * B64).sum()
    S_half = (S_all - D_diag) / 2.0

    n_pairs = (float(n1) * float(n1) - float(n1)) / 2.0

    loss = 0.5 * (S_half + T) / 100.0 / n_pairs
    return np.asarray(np.float32(loss))


# revision 20
# speedup vs baseline: 1.4924x; 1.0761x over previous
"""Trainium2 Bass kernel for the pairwise concordance-index loss.

reference:
    loss = sum_{i<j, f_i=f_j=1} relu((p_i-p_j)(t_i-t_j)) / 100 / n_pairs

Math:
  Only flagged (f=1) entries contribute, so the host first COMPACTS the
  arrays to the n1 flagged entries (padded with zeros to NB*128), which
  shrinks the pairwise matrix from B^2 to ~(0.7B)^2.
  M[i,j] = (p_i-p_j)(t_i-t_j) = A^T B, rank 4:
      A = [u, 1, p, t],  B = [1, u, -t, -p],  u = p*t   (zeros in padding)
  sum relu(M) = 0.5*(sum M + sum |M|); sum M has an O(n) closed form done
  on the host in fp64; sum |M| is the O(n^2) part done on device.

Device decomposition (8 cores, identical program, data-sharded):
  NB row-blocks of 128 rows; core k owns NBC=NB/8 blocks as a quad gang
  (4 blocks) + duo gang (NBC-4). Each block processes cyclic column
  offsets e=0..NB/2 (cols 128(a+e) mod NB*128): e=1..NB/2-1 at weight 1;
  e=0 / e=NB/2 at weight 0.5 via 0.5-pre-scaled slab appendices
  (host-side), so all device sums have uniform weight.

Device structure (raw Bass, hand-rolled semaphores — no TileContext, so
the multi-microsecond scheduler prologue/epilogue is avoided):
  PE: per 2-bank PSUM tile, 2-4 K=4 bf16 matmuls packed into disjoint
  32-row PE groups via tile_position (concurrent matmuls always target
  distinct PSUM banks). Each tile is consumed by ONE fused abs-row-sum
  job on the DVE (tensor_reduce(apply_absolute_value)) or the ScalarE
  (activation(Abs, accum_out)), alternating in fill order so both
  engines stream without head-of-line blocking. PSUM is an 8-bank ring
  of four 2-bank buffers; the PE waits on the consumer semaphore before
  reusing a buffer. Input DMAs are split across the sync and scalar
  HWDGE queues, chunked so the first tiles can start early.
"""

import numpy as np

B = 8192
P = 128
NCORE = 8
CUT = 2560

_cache = {}


def _plan(n1):
    """Compile-time plan derived from the flagged count."""
    nb = max(1, -(-n1 // P))        # 128-row blocks needed
    nb = -(-nb // NCORE) * NCORE    # multiple of 8 (even)
    nbc = nb // NCORE               # blocks per core
    eh = nb // 2                    # antipodal offset (weight 1/2)
    mainw = P * (nbc - 1 + eh - 1)  # shared slab for e=1..eh-1
    tailw = nbc * 256               # per-block [e0/2 | e_h/2] appendix
    gangs = [4] * (nbc // 4) + ([nbc % 4] if nbc % 4 else [])
    w = P * (eh - 1)                # main cols per block
    nfull, rem = w // 512, w % 512
    return dict(nb=nb, nbc=nbc, eh=eh, mainw=mainw, tailw=tailw,
                bcols=mainw + tailw, gangs=gangs, nfull=nfull, rem=rem)


def _slots(plan):
    """Interleaved (gang, kind, s) slot list; duo first for early start."""
    gangs = plan["gangs"]
    per_gang = []
    for g, sz in enumerate(gangs):
        sl = [(g, "main", s) for s in range(plan["nfull"])]
        if plan["rem"]:
            sl.append((g, "rem", plan["nfull"]))
        sl.append((g, "tail", 0))
        per_gang.append(sl)
    order = []
    ng = len(per_gang)
    nmax = max(len(s) for s in per_gang)
    for i in range(nmax):
        for g in reversed(range(ng)):  # duo (last gang) first
            if i < len(per_gang[g]):
                order.append(per_gang[g][i])
    return order


def _layout(plan):
    """Ordered tile descriptors: matmul lists, reduce specs, DMA gates."""
    gangs, rem, mainw = plan["gangs"], plan["rem"], plan["mainw"]
    tiles = []
    for g, kind, s in _slots(plan):
        sz = gangs[g]
        off = sum(gangs[:g])
        ntiles = 2 if sz > 2 else 1
        per_tile = [[] for _ in range(ntiles)]
        if kind == "tail":
            # two sequential waves of N=128 so concurrent matmuls (distinct
            # PE row-groups) never share a PSUM bank
            for wave in range(2):
                for q in range(sz):
                    coff = mainw + 256 * (off + q) + 128 * wave
                    ti, bank = (q // 2, q % 2) if ntiles == 2 else (0, q)
                    per_tile[ti].append((q, coff, 128, bank, 128 * wave))
            width = 256
        else:
            width = 512 if kind == "main" else rem
            for q in range(sz):
                coff = P * (off + q) + 512 * s
                ti, bank = (q // 2, q % 2) if ntiles == 2 else (0, q)
                per_tile[ti].append((q, coff, width, bank, 0))
        for ti in range(ntiles):
            mms = per_tile[ti]
            nbank = len(set(b for (_, _, _, b, _) in mms))
            # input chunks this tile's matmuls read (DMA completions are
            # unordered across HWDGE queues, so each chunk gets its own sem)
            chunks = {"a"}
            for (q, coff, n, _b, _c) in mms:
                if coff < CUT:
                    chunks.add(f"b{q}h1")
                if coff + n > CUT:
                    chunks.add(f"b{q}h2")
            tiles.append(dict(acol=P * g, mms=mms, nbank=nbank, width=width,
                              fd=nbank * width, chunks=chunks))
    # engine assignment: alternate DVE/ACT in fill order (keeps both engines
    # streaming); swap one pair near the tail to balance total element count
    n = len(tiles)
    assign = ["V" if i % 2 == 0 else "A" for i in range(n)]
    if n >= 16:
        assign[14], assign[15] = assign[15], assign[14]
    for t, a in zip(tiles, assign):
        t["eng"] = a
    return tiles


def _build(plan):
    """Build + compile the raw Bass module (once per plan)."""
    import concourse.bacc as bacc
    import concourse.mybir as mybir

    f32 = mybir.dt.float32
    bf16 = mybir.dt.bfloat16
    nc = bacc.Bacc("TRN2", target_bir_lowering=False, debug=False,
                   num_devices=NCORE)

    gangs = plan["gangs"]
    bcols = plan["bcols"]
    awidth = P * len(gangs)
    tiles = _layout(plan)
    njobs = len(tiles)
    nV = sum(1 for t in tiles if t["eng"] == "V")

    a_dram = nc.dram_tensor("a_rows", [P, awidth], bf16, kind="ExternalInput")
    b_dram = nc.dram_tensor("b_cols", [4, bcols], bf16, kind="ExternalInput")
    acc_dram = nc.dram_tensor("acc", [P, njobs], f32, kind="ExternalOutput")

    a_sb = nc.alloc_sbuf_tensor("a_sb", [P, awidth], bf16)
    b_sb = nc.alloc_sbuf_tensor("b_sb", [P, bcols], bf16)
    acc_sb = nc.alloc_sbuf_tensor("acc_sb", [P, njobs], f32)
    ps = nc.alloc_psum_tensor("ps", [P, 8, 512], f32)

    sem_mm = nc.alloc_semaphore("sem_mm")    # tiles filled by PE
    sem_v = nc.alloc_semaphore("sem_v")      # DVE jobs done
    sem_a = nc.alloc_semaphore("sem_a")      # ACT jobs done
    sem_out = nc.alloc_semaphore("sem_out")  # output DMA done
    # one sem per input chunk: HWDGE completions are unordered across queues
    chunk_order = ["a", "b0h1", "b1h1", "b2h1", "b3h1",
                   "b0h2", "b1h2", "b2h2", "b3h2"]
    sem_chunk = {c: nc.alloc_semaphore(f"sem_in_{c}") for c in chunk_order}
    sems = [sem_mm, sem_v, sem_a, sem_out] + list(sem_chunk.values())
    nums = sorted(s.num for s in sems)
    assert nums[-1] - nums[0] == len(nums) - 1, nums
    semrange = range(nums[0], nums[-1] + 1)

    # per-tile bookkeeping for sync
    jobidx = {}
    counts = {"V": 0, "A": 0}
    for i, t in enumerate(tiles):
        counts[t["eng"]] += 1
        jobidx[i] = counts[t["eng"]]  # 1-based within its engine

    with nc.Block("k") as blk:

        @blk.sync
        def _(eng):
            # groups 0,2: first-needed halves first
            for q in (0, 2):
                eng.dma_start(b_sb.ap()[32 * q:32 * q + 4, 0:CUT],
                              b_dram.ap()[:, 0:CUT]).then_inc(
                                  sem_chunk[f"b{q}h1"], 16)
            for q in (0, 2):
                eng.dma_start(b_sb.ap()[32 * q:32 * q + 4, CUT:bcols],
                              b_dram.ap()[:, CUT:bcols]).then_inc(
                                  sem_chunk[f"b{q}h2"], 16)

        @blk.tensor
        def _(eng):
            waited = set()
            for i, t in enumerate(tiles):
                for c in chunk_order:
                    if c in t["chunks"] and c not in waited:
                        waited.add(c)
                        eng.wait_ge(sem_chunk[c], 16)
                if i >= 4:
                    p = i - 4  # previous occupant of this 2-bank buffer
                    eng.wait_ge(sem_v if tiles[p]["eng"] == "V" else sem_a,
                                jobidx[p])
                buf = i % 4
                last = len(t["mms"]) - 1
                for j, (q, coff, n, bank, c0) in enumerate(t["mms"]):
                    ins = nc.tensor.matmul(
                        ps.ap()[:, 2 * buf + bank, c0:c0 + n],
                        a_sb.ap()[32 * q:32 * q + 4, t["acol"]:t["acol"] + P],
                        b_sb.ap()[32 * q:32 * q + 4, coff:coff + n],
                        start=True,
                        stop=True,
                        tile_position=(32 * q, 0),
                    )
                    if j == last:
                        ins.then_inc(sem_mm, 1)

        @blk.vector
        def _(eng):
            for i, t in enumerate(tiles):
                if t["eng"] != "V":
                    continue
                buf = i % 4
                eng.wait_ge(sem_mm, i + 1)
                eng.tensor_reduce(
                    acc_sb.ap()[:, i:i + 1],
                    ps.ap()[:, 2 * buf:2 * buf + t["nbank"], 0:t["width"]],
                    axis=mybir.AxisListType.XY, op=mybir.AluOpType.add,
                    apply_absolute_value=True,
                ).then_inc(sem_v, 1)

        @blk.scalar
        def _(eng):
            eng.dma_start(a_sb.ap()[:, :], a_dram.ap()[:, :]).then_inc(
                sem_chunk["a"], 16)
            for q in (1, 3):
                eng.dma_start(b_sb.ap()[32 * q:32 * q + 4, 0:CUT],
                              b_dram.ap()[:, 0:CUT]).then_inc(
                                  sem_chunk[f"b{q}h1"], 16)
            for q in (1, 3):
                eng.dma_start(b_sb.ap()[32 * q:32 * q + 4, CUT:bcols],
                              b_dram.ap()[:, CUT:bcols]).then_inc(
                                  sem_chunk[f"b{q}h2"], 16)
            for i, t in enumerate(tiles):
                if t["eng"] != "A":
                    continue
                buf = i % 4
                eng.wait_ge(sem_mm, i + 1)
                red = ps.ap()[:, 2 * buf:2 * buf + t["nbank"], 0:t["width"]]
                eng.activation(
                    red, red,
                    mybir.ActivationFunctionType.Abs,
                    accum_out=acc_sb.ap()[:, i:i + 1],
                ).then_inc(sem_a, 1)
            # explicit edges: ACT's own accumulator writes + DVE columns
            eng.wait_ge(sem_a, njobs - nV)
            eng.wait_ge(sem_v, nV)
            eng.dma_start(acc_dram.ap()[:, :], acc_sb.ap()[:, :]).then_inc(
                sem_out, 16)
            eng.wait_ge(sem_out, 16)

    # reset semaphores so repeated executions of this NEFF start clean;
    # the block exit drained every engine, the barrier orders the clear
    # after all of them (each sem's final value was waited in-program).
    nc.all_engine_barrier()
    nc.gpsimd.dma_reset(semrange)
    nc.gpsimd.sem_clear(semrange)
    nc.all_engine_barrier()

    nc.compile()
    return nc


def _get_nc(plan):
    key = ("nc", plan["nb"])
    if key not in _cache:
        _cache[key] = _build(plan)
    return _cache[key]


def _prepare(pred, gt, ift, imf):
    """Compact + pad + build per-core input maps."""
    import ml_dtypes

    p_full = np.asarray(pred).astype(np.float32)
    gt = np.asarray(gt).astype(np.float32)
    t_full = gt[:, ift]
    f_full = gt[:, imf] == 1
    idx = np.flatnonzero(f_full)
    n1 = len(idx)

    plan = _plan(n1)
    npad = plan["nb"] * P
    p = np.zeros(npad, np.float32)
    t = np.zeros(npad, np.float32)
    w = np.zeros(npad, np.float32)
    p[:n1] = p_full[idx]
    t[:n1] = t_full[idx]
    w[:n1] = 1.0
    u = p * t

    # compaction makes the flags trivial: real entries are all flagged,
    # padded entries are exactly zero in every factor.
    A = np.ascontiguousarray(
        np.stack([u, w, p, t]).astype(ml_dtypes.bfloat16)
    )
    Bm = np.ascontiguousarray(
        np.stack([w, u, -t, -p]).astype(ml_dtypes.bfloat16)
    )
    Bh = Bm * np.asarray(0.5, dtype=ml_dtypes.bfloat16)  # exact halving

    nbc, eh, mainw = plan["nbc"], plan["eh"], plan["mainw"]
    gangs = plan["gangs"]
    awidth = P * len(gangs)
    in_maps = []
    for k in range(NCORE):
        a_rows = np.zeros((P, awidth), dtype=ml_dtypes.bfloat16)
        for g, sz in enumerate(gangs):
            off = sum(gangs[:g])
            for q in range(sz):
                blk = nbc * k + off + q
                a_rows[32 * q:32 * q + 4, P * g:P * g + P] = \
                    A[:, P * blk:P * blk + P]

        b_colsk = np.empty((4, plan["bcols"]), dtype=ml_dtypes.bfloat16)
        cols = (P * (nbc * k + 1) + np.arange(mainw)) % npad
        b_colsk[:, 0:mainw] = Bm[:, cols]
        for a in range(nbc):
            c0 = (P * (nbc * k + a) + np.arange(P)) % npad
            ch = (P * (nbc * k + a + eh) + np.arange(P)) % npad
            base = mainw + 256 * a
            b_colsk[:, base:base + P] = Bh[:, c0]
            b_colsk[:, base + P:base + 256] = Bh[:, ch]
        in_maps.append(
            {"a_rows": a_rows, "b_cols": np.ascontiguousarray(b_colsk)}
        )
    return in_maps, A, Bm, n1, plan


def kernel(pred, gt, gt_fracTime, gt_ifMOF):
    from concourse import bass_utils

    ift = int(np.asarray(gt_fracTime))
    imf = int(np.asarray(gt_ifMOF))

    in_maps, A, Bm, n1, plan = _prepare(pred, gt, ift, imf)
    nc = _get_nc(plan)
    res = bass_utils.run_bass_kernel_spmd(nc, in_maps,
                                          core_ids=list(range(NCORE)))

    # T = sum_{i<j} |M| (all device accumulator columns are weight 1)
    T = 0.0
    for r in res.results:
        T += r["acc"].astype(np.float64).sum()

    # host closed form in fp64 over the same bf16 values the device used:
    # sum_{i<j} M = (sum_{i,j} M - sum_diag M) / 2
    A64 = A.astype(np.float64)
    B64 = Bm.astype(np.float64)
    S_all = (A64.sum(axis=1) * B64.sum(axis=1)).sum()
    D_diag = (A64 * B64).sum()
    S_half = (S_all - D_diag) / 2.0

    n_pairs = (float(n1) * float(n1) - float(n1)) / 2.0

    loss = 0.5 * (S_half + T) / 100.0 / n_pairs
    return np.asarray(np.float32(loss))


# revision 23
# speedup vs baseline: 1.5288x; 1.0243x over previous
"""Trainium2 Bass kernel for the pairwise concordance-index loss.

reference:
    loss = sum_{i<j, f_i=f_j=1} relu((p_i-p_j)(t_i-t_j)) / 100 / n_pairs

Math:
  Only flagged (f=1) entries contribute, so the host first COMPACTS the
  arrays to the n1 flagged entries (padded with zeros to NB*128), which
  shrinks the pairwise matrix from B^2 to ~(0.7B)^2.
  M[i,j] = (p_i-p_j)(t_i-t_j) = A^T B, rank 4:
      A = [u, 1, p, t],  B = [1, u, -t, -p],  u = p*t   (zeros in padding)
  sum relu(M) = 0.5*(sum M + sum |M|); sum M has an O(n) closed form done
  on the host in fp64; sum |M| is the O(n^2) part done on device.

Device decomposition (8 cores, identical program, data-sharded):
  NB row-blocks of 128 rows; core k owns NBC=NB/8 blocks as a quad gang
  (4 blocks) + duo gang (NBC-4). Each block processes cyclic column
  offsets e=0..NB/2 (cols 128(a+e) mod NB*128): e=1..NB/2-1 at weight 1;
  e=0 / e=NB/2 at weight 0.5 via 0.5-pre-scaled slab appendices
  (host-side), so all device sums have uniform weight.

Device structure (raw Bass, hand-rolled semaphores — no TileContext, so
the multi-microsecond scheduler prologue/epilogue is avoided):
  PE: per 2-bank PSUM tile, 2-4 K=4 bf16 matmuls packed into disjoint
  32-row PE groups via tile_position (concurrent matmuls always target
  distinct PSUM banks). Each tile is consumed by ONE fused abs-row-sum
  job on the DVE (tensor_reduce(apply_absolute_value)) or the ScalarE
  (activation(Abs, accum_out)), alternating in fill order so both
  engines stream without head-of-line blocking. PSUM is an 8-bank ring
  of four 2-bank buffers; the PE waits on the consumer semaphore before
  reusing a buffer. Input DMAs are split across the sync and scalar
  HWDGE queues, chunked so the first tiles can start early.
"""

import numpy as np

B = 8192
P = 128
NCORE = 8
CUT = 2560

_cache = {}


def _plan(n1):
    """Compile-time plan derived from the flagged count."""
    nb = max(1, -(-n1 // P))        # 128-row blocks needed
    nb = -(-nb // NCORE) * NCORE    # multiple of 8 (even)
    nbc = nb // NCORE               # blocks per core
    eh = nb // 2                    # antipodal offset (weight 1/2)
    mainw = P * (nbc - 1 + eh - 1)  # shared slab for e=1..eh-1
    tailw = nbc * 256               # per-block [e0/2 | e_h/2] appendix
    gangs = [4] * (nbc // 4) + ([nbc % 4] if nbc % 4 else [])
    w = P * (eh - 1)                # main cols per block
    nfull, rem = w // 512, w % 512
    return dict(nb=nb, nbc=nbc, eh=eh, mainw=mainw, tailw=tailw,
                bcols=mainw + tailw, gangs=gangs, nfull=nfull, rem=rem)


def _slots(plan):
    """Interleaved (gang, kind, s) slot list; duo first for early start."""
    gangs = plan["gangs"]
    per_gang = []
    for g, sz in enumerate(gangs):
        sl = [(g, "main", s) for s in range(plan["nfull"])]
        if plan["rem"]:
            sl.append((g, "rem", plan["nfull"]))
        sl.append((g, "tail", 0))
        per_gang.append(sl)
    order = []
    ng = len(per_gang)
    nmax = max(len(s) for s in per_gang)
    for i in range(nmax):
        for g in reversed(range(ng)):  # duo (last gang) first
            if i < len(per_gang[g]):
                order.append(per_gang[g][i])
    return order


def _layout(plan):
    """Ordered tile descriptors: matmul lists, reduce specs, DMA gates."""
    gangs, rem, mainw = plan["gangs"], plan["rem"], plan["mainw"]
    tiles = []
    for g, kind, s in _slots(plan):
        sz = gangs[g]
        off = sum(gangs[:g])
        ntiles = 2 if sz > 2 else 1
        per_tile = [[] for _ in range(ntiles)]
        if kind == "tail":
            # two sequential waves of N=128 so concurrent matmuls (distinct
            # PE row-groups) never share a PSUM bank
            for wave in range(2):
                for q in range(sz):
                    coff = mainw + 256 * (off + q) + 128 * wave
                    ti, bank = (q // 2, q % 2) if ntiles == 2 else (0, q)
                    per_tile[ti].append((q, coff, 128, bank, 128 * wave))
            width = 256
        else:
            width = 512 if kind == "main" else rem
            for q in range(sz):
                coff = P * (off + q) + 512 * s
                ti, bank = (q // 2, q % 2) if ntiles == 2 else (0, q)
                per_tile[ti].append((q, coff, width, bank, 0))
        for ti in range(ntiles):
            mms = per_tile[ti]
            nbank = len(set(b for (_, _, _, b, _) in mms))
            # input chunks this tile's matmuls read (DMA completions are
            # unordered across HWDGE queues, so each chunk gets its own sem)
            chunks = {"a"}
            for (q, coff, n, _b, _c) in mms:
                if coff < CUT:
                    chunks.add(f"b{q}h1")
                if coff + n > CUT:
                    chunks.add(f"b{q}h2")
            tiles.append(dict(acol=P * g, mms=mms, nbank=nbank, width=width,
                              fd=nbank * width, chunks=chunks))
    # engine assignment: alternate DVE/ACT in fill order (keeps both engines
    # streaming); swap one pair near the tail to balance total element count
    n = len(tiles)
    assign = ["V" if i % 2 == 0 else "A" for i in range(n)]
    if n >= 16:
        assign[14], assign[15] = assign[15], assign[14]
    for t, a in zip(tiles, assign):
        t["eng"] = a
    return tiles


def _build(plan):
    """Build + compile the raw Bass module (once per plan)."""
    import concourse.bacc as bacc
    import concourse.mybir as mybir

    f32 = mybir.dt.float32
    bf16 = mybir.dt.bfloat16
    nc = bacc.Bacc("TRN2", target_bir_lowering=False, debug=False,
                   num_devices=NCORE)

    gangs = plan["gangs"]
    bcols = plan["bcols"]
    awidth = P * len(gangs)
    tiles = _layout(plan)
    njobs = len(tiles)
    nV = sum(1 for t in tiles if t["eng"] == "V")

    a_dram = nc.dram_tensor("a_rows", [P, awidth], bf16, kind="ExternalInput")
    b_dram = nc.dram_tensor("b_cols", [4, bcols], bf16, kind="ExternalInput")
    acc_dram = nc.dram_tensor("acc", [P, njobs], f32, kind="ExternalOutput")

    a_sb = nc.alloc_sbuf_tensor("a_sb", [P, awidth], bf16)
    b_sb = nc.alloc_sbuf_tensor("b_sb", [P, bcols], bf16)
    acc_sb = nc.alloc_sbuf_tensor("acc_sb", [P, njobs], f32)
    ps = nc.alloc_psum_tensor("ps", [P, 8, 512], f32)

    sem_mm = nc.alloc_semaphore("sem_mm")    # tiles filled by PE
    sem_v = nc.alloc_semaphore("sem_v")      # DVE jobs done
    sem_a = nc.alloc_semaphore("sem_a")      # ACT jobs done
    sem_out = nc.alloc_semaphore("sem_out")  # output DMA done
    # one sem per input chunk: HWDGE completions are unordered across queues
    chunk_order = ["a", "b0h1", "b1h1", "b2h1", "b3h1",
                   "b0h2", "b1h2", "b2h2", "b3h2"]
    sem_chunk = {c: nc.alloc_semaphore(f"sem_in_{c}") for c in chunk_order}
    sems = [sem_mm, sem_v, sem_a, sem_out] + list(sem_chunk.values())
    nums = sorted(s.num for s in sems)
    assert nums[-1] - nums[0] == len(nums) - 1, nums
    semrange = range(nums[0], nums[-1] + 1)

    # per-tile bookkeeping for sync
    jobidx = {}
    counts = {"V": 0, "A": 0}
    for i, t in enumerate(tiles):
        counts[t["eng"]] += 1
        jobidx[i] = counts[t["eng"]]  # 1-based within its engine

    with nc.Block("k") as blk:

        @blk.sync
        def _(eng):
            # groups 0,2: first-needed halves first
            for q in (0, 2):
                eng.dma_start(b_sb.ap()[32 * q:32 * q + 4, 0:CUT],
                              b_dram.ap()[:, 0:CUT]).then_inc(
                                  sem_chunk[f"b{q}h1"], 16)
            for q in (0, 2, 3):
                eng.dma_start(b_sb.ap()[32 * q:32 * q + 4, CUT:bcols],
                              b_dram.ap()[:, CUT:bcols]).then_inc(
                                  sem_chunk[f"b{q}h2"], 16)

        @blk.tensor
        def _(eng):
            waited = set()
            for i, t in enumerate(tiles):
                for c in chunk_order:
                    if c in t["chunks"] and c not in waited:
                        waited.add(c)
                        eng.wait_ge(sem_chunk[c], 16)
                if i >= 4:
                    p = i - 4  # previous occupant of this 2-bank buffer
                    eng.wait_ge(sem_v if tiles[p]["eng"] == "V" else sem_a,
                                jobidx[p])
                buf = i % 4
                last = len(t["mms"]) - 1
                for j, (q, coff, n, bank, c0) in enumerate(t["mms"]):
                    ins = nc.tensor.matmul(
                        ps.ap()[:, 2 * buf + bank, c0:c0 + n],
                        a_sb.ap()[32 * q:32 * q + 4, t["acol"]:t["acol"] + P],
                        b_sb.ap()[32 * q:32 * q + 4, coff:coff + n],
                        start=True,
                        stop=True,
                        tile_position=(32 * q, 0),
                    )
                    if j == last:
                        ins.then_inc(sem_mm, 1)

        @blk.vector
        def _(eng):
            for i, t in enumerate(tiles):
                if t["eng"] != "V":
                    continue
                buf = i % 4
                eng.wait_ge(sem_mm, i + 1)
                eng.tensor_reduce(
                    acc_sb.ap()[:, i:i + 1],
                    ps.ap()[:, 2 * buf:2 * buf + t["nbank"], 0:t["width"]],
                    axis=mybir.AxisListType.XY, op=mybir.AluOpType.add,
                    apply_absolute_value=True,
                ).then_inc(sem_v, 1)

        @blk.scalar
        def _(eng):
            eng.dma_start(a_sb.ap()[:, :], a_dram.ap()[:, :]).then_inc(
                sem_chunk["a"], 16)
            for q in (1, 3):
                eng.dma_start(b_sb.ap()[32 * q:32 * q + 4, 0:CUT],
                              b_dram.ap()[:, 0:CUT]).then_inc(
                                  sem_chunk[f"b{q}h1"], 16)
            eng.dma_start(b_sb.ap()[32 * 1:32 * 1 + 4, CUT:bcols],
                          b_dram.ap()[:, CUT:bcols]).then_inc(
                              sem_chunk["b1h2"], 16)
            for i, t in enumerate(tiles):
                if t["eng"] != "A":
                    continue
                buf = i % 4
                eng.wait_ge(sem_mm, i + 1)
                red = ps.ap()[:, 2 * buf:2 * buf + t["nbank"], 0:t["width"]]
                eng.activation(
                    red, red,
                    mybir.ActivationFunctionType.Abs,
                    accum_out=acc_sb.ap()[:, i:i + 1],
                ).then_inc(sem_a, 1)
            # explicit edges: ACT's own accumulator writes + DVE columns
            eng.wait_ge(sem_a, njobs - nV)
            eng.wait_ge(sem_v, nV)
            eng.dma_start(acc_dram.ap()[:, :], acc_sb.ap()[:, :]).then_inc(
                sem_out, 16)
            eng.wait_ge(sem_out, 16)

    # reset semaphores so repeated executions of this NEFF start clean;
    # the block exit drained every engine, the barrier orders the clear
    # after all of them (each sem's final value was waited in-program).
    nc.all_engine_barrier()
    nc.gpsimd.dma_reset(semrange)
    nc.gpsimd.sem_clear(semrange)

    nc.compile()
    return nc


def _get_nc(plan):
    key = ("nc", plan["nb"])
    if key not in _cache:
        _cache[key] = _build(plan)
    return _cache[key]


def _prepare(pred, gt, ift, imf):
    """Compact + pad + build per-core input maps."""
    import ml_dtypes

    p_full = np.asarray(pred).astype(np.float32)
    gt = np.asarray(gt).astype(np.float32)
    t_full = gt[:, ift]
    f_full = gt[:, imf] == 1
    idx = np.flatnonzero(f_full)
    n1 = len(idx)

    plan = _plan(n1)
    npad = plan["nb"] * P
    p = np.zeros(npad, np.float32)
    t = np.zeros(npad, np.float32)
    w = np.zeros(npad, np.float32)
    p[:n1] = p_full[idx]
    t[:n1] = t_full[idx]
    w[:n1] = 1.0
    u = p * t

    # compaction makes the flags trivial: real entries are all flagged,
    # padded entries are exactly zero in every factor.
    A = np.ascontiguousarray(
        np.stack([u, w, p, t]).astype(ml_dtypes.bfloat16)
    )
    Bm = np.ascontiguousarray(
        np.stack([w, u, -t, -p]).astype(ml_dtypes.bfloat16)
    )
    Bh = Bm * np.asarray(0.5, dtype=ml_dtypes.bfloat16)  # exact halving

    nbc, eh, mainw = plan["nbc"], plan["eh"], plan["mainw"]
    gangs = plan["gangs"]
    awidth = P * len(gangs)
    in_maps = []
    for k in range(NCORE):
        a_rows = np.zeros((P, awidth), dtype=ml_dtypes.bfloat16)
        for g, sz in enumerate(gangs):
            off = sum(gangs[:g])
            for q in range(sz):
                blk = nbc * k + off + q
                a_rows[32 * q:32 * q + 4, P * g:P * g + P] = \
                    A[:, P * blk:P * blk + P]

        b_colsk = np.empty((4, plan["bcols"]), dtype=ml_dtypes.bfloat16)
        cols = (P * (nbc * k + 1) + np.arange(mainw)) % npad
        b_colsk[:, 0:mainw] = Bm[:, cols]
        for a in range(nbc):
            c0 = (P * (nbc * k + a) + np.arange(P)) % npad
            ch = (P * (nbc * k + a + eh) + np.arange(P)) % npad
            base = mainw + 256 * a
            b_colsk[:, base:base + P] = Bh[:, c0]
            b_colsk[:, base + P:base + 256] = Bh[:, ch]
        in_maps.append(
            {"a_rows": a_rows, "b_cols": np.ascontiguousarray(b_colsk)}
        )
    return in_maps, A, Bm, n1, plan


def kernel(pred, gt, gt_fracTime, gt_ifMOF):
    from concourse import bass_utils

    ift = int(np.asarray(gt_fracTime))
    imf = int(np.asarray(gt_ifMOF))

    in_maps, A, Bm, n1, plan = _prepare(pred, gt, ift, imf)
    nc = _get_nc(plan)
    res = bass_utils.run_bass_kernel_spmd(nc, in_maps,
                                          core_ids=list(range(NCORE)))

    # T = sum_{i<j} |M| (all device accumulator columns are weight 1)
    T = 0.0
    for r in res.results:
        T += r["acc"].astype(np.float64).sum()

    # host closed form in fp64 over the same bf16 values the device used:
    # sum_{i<j} M = (sum_{i,j} M - sum_diag M) / 2
    A64 = A.astype(np.float64)
    B64 = Bm.astype(np.float64)
    S_all = (A64.sum(axis=1) * B64.sum(axis=1)).sum()
    D_diag = (A64 * B64).sum()
    S_half = (S_all - D_diag) / 2.0

    n_pairs = (float(n1) * float(n1) - float(n1)) / 2.0

    loss = 0.5 * (S_half + T) / 100.0 / n_pairs
    return np.asarray(np.float32(loss))


# revision 25
# speedup vs baseline: 1.5308x; 1.0013x over previous
"""Trainium2 Bass kernel for the pairwise concordance-index loss.

reference:
    loss = sum_{i<j, f_i=f_j=1} relu((p_i-p_j)(t_i-t_j)) / 100 / n_pairs

Math:
  Only flagged (f=1) entries contribute, so the host first COMPACTS the
  arrays to the n1 flagged entries (padded with zeros to NB*128), which
  shrinks the pairwise matrix from B^2 to ~(0.7B)^2.
  M[i,j] = (p_i-p_j)(t_i-t_j) = A^T B, rank 4:
      A = [u, 1, p, t],  B = [1, u, -t, -p],  u = p*t   (zeros in padding)
  sum relu(M) = 0.5*(sum M + sum |M|); sum M has an O(n) closed form done
  on the host in fp64; sum |M| is the O(n^2) part done on device.

Device decomposition (8 cores, identical program, data-sharded):
  NB row-blocks of 128 rows; core k owns NBC=NB/8 blocks as a quad gang
  (4 blocks) + duo gang (NBC-4). Each block processes cyclic column
  offsets e=0..NB/2 (cols 128(a+e) mod NB*128): e=1..NB/2-1 at weight 1;
  e=0 / e=NB/2 at weight 0.5 via 0.5-pre-scaled slab appendices
  (host-side), so all device sums have uniform weight.

Device structure (raw Bass, hand-rolled semaphores — no TileContext, so
the multi-microsecond scheduler prologue/epilogue is avoided):
  PE: per 2-bank PSUM tile, 2-4 K=4 bf16 matmuls packed into disjoint
  32-row PE groups via tile_position (concurrent matmuls always target
  distinct PSUM banks). Each tile is consumed by ONE fused abs-row-sum
  job on the DVE (tensor_reduce(apply_absolute_value)) or the ScalarE
  (activation(Abs, accum_out)), alternating in fill order so both
  engines stream without head-of-line blocking. PSUM is an 8-bank ring
  of four 2-bank buffers; the PE waits on the consumer semaphore before
  reusing a buffer. Input DMAs are split across the sync and scalar
  HWDGE queues, chunked so the first tiles can start early.
"""

import numpy as np

B = 8192
P = 128
NCORE = 8
CUT = 2560

_cache = {}


def _plan(n1):
    """Compile-time plan derived from the flagged count."""
    nb = max(1, -(-n1 // P))        # 128-row blocks needed
    nb = -(-nb // NCORE) * NCORE    # multiple of 8 (even)
    nbc = nb // NCORE               # blocks per core
    eh = nb // 2                    # antipodal offset (weight 1/2)
    mainw = P * (nbc - 1 + eh - 1)  # shared slab for e=1..eh-1
    tailw = nbc * 256               # per-block [e0/2 | e_h/2] appendix
    gangs = [4] * (nbc // 4) + ([nbc % 4] if nbc % 4 else [])
    w = P * (eh - 1)                # main cols per block
    nfull, rem = w // 512, w % 512
    return dict(nb=nb, nbc=nbc, eh=eh, mainw=mainw, tailw=tailw,
                bcols=mainw + tailw, gangs=gangs, nfull=nfull, rem=rem)


def _slots(plan):
    """Interleaved (gang, kind, s) slot list; duo first for early start."""
    gangs = plan["gangs"]
    per_gang = []
    for g, sz in enumerate(gangs):
        sl = [(g, "main", s) for s in range(plan["nfull"])]
        if plan["rem"]:
            sl.append((g, "rem", plan["nfull"]))
        sl.append((g, "tail", 0))
        per_gang.append(sl)
    order = []
    ng = len(per_gang)
    nmax = max(len(s) for s in per_gang)
    for i in range(nmax):
        for g in reversed(range(ng)):  # duo (last gang) first
            if i < len(per_gang[g]):
                order.append(per_gang[g][i])
    return order


def _layout(plan):
    """Ordered tile descriptors: matmul lists, reduce specs, DMA gates."""
    gangs, rem, mainw = plan["gangs"], plan["rem"], plan["mainw"]
    tiles = []
    for g, kind, s in _slots(plan):
        sz = gangs[g]
        off = sum(gangs[:g])
        ntiles = 2 if sz > 2 else 1
        per_tile = [[] for _ in range(ntiles)]
        if kind == "tail":
            # two sequential waves of N=128 so concurrent matmuls (distinct
            # PE row-groups) never share a PSUM bank
            for wave in range(2):
                for q in range(sz):
                    coff = mainw + 256 * (off + q) + 128 * wave
                    ti, bank = (q // 2, q % 2) if ntiles == 2 else (0, q)
                    per_tile[ti].append((q, coff, 128, bank, 128 * wave))
            width = 256
        else:
            width = 512 if kind == "main" else rem
            for q in range(sz):
                coff = P * (off + q) + 512 * s
                ti, bank = (q // 2, q % 2) if ntiles == 2 else (0, q)
                per_tile[ti].append((q, coff, width, bank, 0))
        for ti in range(ntiles):
            mms = per_tile[ti]
            nbank = len(set(b for (_, _, _, b, _) in mms))
            # input chunks this tile's matmuls read (DMA completions are
            # unordered across HWDGE queues, so each chunk gets its own sem)
            chunks = {"a"}
            for (q, coff, n, _b, _c) in mms:
                if coff < CUT:
                    chunks.add(f"b{q}h1")
                if coff + n > CUT:
                    chunks.add(f"b{q}h2")
            tiles.append(dict(acol=P * g, mms=mms, nbank=nbank, width=width,
                              fd=nbank * width, chunks=chunks))
    # engine assignment: alternate DVE/ACT in fill order (keeps both engines
    # streaming); swap one pair near the tail to balance total element count
    n = len(tiles)
    assign = ["V" if i % 2 == 0 else "A" for i in range(n)]
    if n >= 16:
        assign[14], assign[15] = assign[15], assign[14]
    for t, a in zip(tiles, assign):
        t["eng"] = a
    return tiles


def _build(plan):
    """Build + compile the raw Bass module (once per plan)."""
    import concourse.bacc as bacc
    import concourse.mybir as mybir

    f32 = mybir.dt.float32
    bf16 = mybir.dt.bfloat16
    nc = bacc.Bacc("TRN2", target_bir_lowering=False, debug=False,
                   num_devices=NCORE)

    gangs = plan["gangs"]
    bcols = plan["bcols"]
    awidth = P * len(gangs)
    tiles = _layout(plan)
    njobs = len(tiles)
    nV = sum(1 for t in tiles if t["eng"] == "V")

    a_dram = nc.dram_tensor("a_rows", [P, awidth], bf16, kind="ExternalInput")
    b_dram = nc.dram_tensor("b_cols", [4, bcols], bf16, kind="ExternalInput")
    acc_dram = nc.dram_tensor("acc", [P, njobs], f32, kind="ExternalOutput")

    a_sb = nc.alloc_sbuf_tensor("a_sb", [P, awidth], bf16)
    b_sb = nc.alloc_sbuf_tensor("b_sb", [P, bcols], bf16)
    acc_sb = nc.alloc_sbuf_tensor("acc_sb", [P, njobs], f32)
    ps = nc.alloc_psum_tensor("ps", [P, 8, 512], f32)

    sem_mm = nc.alloc_semaphore("sem_mm")    # tiles filled by PE
    sem_v = nc.alloc_semaphore("sem_v")      # DVE jobs done
    sem_a = nc.alloc_semaphore("sem_a")      # ACT jobs done
    sem_out = nc.alloc_semaphore("sem_out")  # output DMA done
    # one sem per input chunk: HWDGE completions are unordered across queues
    chunk_order = ["a", "b0h1", "b1h1", "b2h1", "b3h1",
                   "b0h2", "b1h2", "b2h2", "b3h2"]
    sem_chunk = {c: nc.alloc_semaphore(f"sem_in_{c}") for c in chunk_order}
    sems = [sem_mm, sem_v, sem_a, sem_out] + list(sem_chunk.values())
    nums = sorted(s.num for s in sems)
    assert nums[-1] - nums[0] == len(nums) - 1, nums
    semrange = range(nums[0], nums[-1] + 1)

    # per-tile bookkeeping for sync
    jobidx = {}
    counts = {"V": 0, "A": 0}
    for i, t in enumerate(tiles):
        counts[t["eng"]] += 1
        jobidx[i] = counts[t["eng"]]  # 1-based within its engine

    with nc.Block("k") as blk:

        @blk.sync
        def _(eng):
            # everything the first tiles need goes on this (faster) queue,
            # most-critical first
            for q in (0, 1):
                eng.dma_start(b_sb.ap()[32 * q:32 * q + 4, 0:CUT],
                              b_dram.ap()[:, 0:CUT]).then_inc(
                                  sem_chunk[f"b{q}h1"], 16)
            eng.dma_start(a_sb.ap()[:, :], a_dram.ap()[:, :]).then_inc(
                sem_chunk["a"], 16)
            eng.dma_start(b_sb.ap()[32 * 2:32 * 2 + 4, 0:CUT],
                          b_dram.ap()[:, 0:CUT]).then_inc(
                              sem_chunk["b2h1"], 16)
            for q in (0, 2):
                eng.dma_start(b_sb.ap()[32 * q:32 * q + 4, CUT:bcols],
                              b_dram.ap()[:, CUT:bcols]).then_inc(
                                  sem_chunk[f"b{q}h2"], 16)

        @blk.tensor
        def _(eng):
            waited = set()
            for i, t in enumerate(tiles):
                for c in chunk_order:
                    if c in t["chunks"] and c not in waited:
                        waited.add(c)
                        eng.wait_ge(sem_chunk[c], 16)
                if i >= 4:
                    p = i - 4  # previous occupant of this 2-bank buffer
                    eng.wait_ge(sem_v if tiles[p]["eng"] == "V" else sem_a,
                                jobidx[p])
                buf = i % 4
                last = len(t["mms"]) - 1
                for j, (q, coff, n, bank, c0) in enumerate(t["mms"]):
                    ins = nc.tensor.matmul(
                        ps.ap()[:, 2 * buf + bank, c0:c0 + n],
                        a_sb.ap()[32 * q:32 * q + 4, t["acol"]:t["acol"] + P],
                        b_sb.ap()[32 * q:32 * q + 4, coff:coff + n],
                        start=True,
                        stop=True,
                        tile_position=(32 * q, 0),
                    )
                    if j == last:
                        ins.then_inc(sem_mm, 1)

        @blk.vector
        def _(eng):
            for i, t in enumerate(tiles):
                if t["eng"] != "V":
                    continue
                buf = i % 4
                eng.wait_ge(sem_mm, i + 1)
                eng.tensor_reduce(
                    acc_sb.ap()[:, i:i + 1],
                    ps.ap()[:, 2 * buf:2 * buf + t["nbank"], 0:t["width"]],
                    axis=mybir.AxisListType.XY, op=mybir.AluOpType.add,
                    apply_absolute_value=True,
                ).then_inc(sem_v, 1)

        @blk.scalar
        def _(eng):
            eng.dma_start(b_sb.ap()[32 * 3:32 * 3 + 4, 0:CUT],
                          b_dram.ap()[:, 0:CUT]).then_inc(
                              sem_chunk["b3h1"], 16)
            for q in (1, 3):
                eng.dma_start(b_sb.ap()[32 * q:32 * q + 4, CUT:bcols],
                              b_dram.ap()[:, CUT:bcols]).then_inc(
                                  sem_chunk[f"b{q}h2"], 16)
            for i, t in enumerate(tiles):
                if t["eng"] != "A":
                    continue
                buf = i % 4
                eng.wait_ge(sem_mm, i + 1)
                red = ps.ap()[:, 2 * buf:2 * buf + t["nbank"], 0:t["width"]]
                eng.activation(
                    red, red,
                    mybir.ActivationFunctionType.Abs,
                    accum_out=acc_sb.ap()[:, i:i + 1],
                ).then_inc(sem_a, 1)
            # explicit edges: ACT's own accumulator writes + DVE columns
            eng.wait_ge(sem_a, njobs - nV)
            eng.wait_ge(sem_v, nV)
            eng.dma_start(acc_dram.ap()[:, :], acc_sb.ap()[:, :]).then_inc(
                sem_out, 16)
            eng.wait_ge(sem_out, 16)

    # reset semaphores so repeated executions of this NEFF start clean;
    # the block exit drained every engine, the barrier orders the clear
    # after all of them (each sem's final value was waited in-program).
    nc.all_engine_barrier()
    nc.gpsimd.dma_reset(semrange)
    nc.gpsimd.sem_clear(semrange)

    nc.compile()
    return nc


def _get_nc(plan):
    key = ("nc", plan["nb"])
    if key not in _cache:
        _cache[key] = _build(plan)
    return _cache[key]


def _prepare(pred, gt, ift, imf):
    """Compact + pad + build per-core input maps."""
    import ml_dtypes

    p_full = np.asarray(pred).astype(np.float32)
    gt = np.asarray(gt).astype(np.float32)
    t_full = gt[:, ift]
    f_full = gt[:, imf] == 1
    idx = np.flatnonzero(f_full)
    n1 = len(idx)

    plan = _plan(n1)
    npad = plan["nb"] * P
    p = np.zeros(npad, np.float32)
    t = np.zeros(npad, np.float32)
    w = np.zeros(npad, np.float32)
    p[:n1] = p_full[idx]
    t[:n1] = t_full[idx]
    w[:n1] = 1.0
    u = p * t

    # compaction makes the flags trivial: real entries are all flagged,
    # padded entries are exactly zero in every factor.
    A = np.ascontiguousarray(
        np.stack([u, w, p, t]).astype(ml_dtypes.bfloat16)
    )
    Bm = np.ascontiguousarray(
        np.stack([w, u, -t, -p]).astype(ml_dtypes.bfloat16)
    )
    Bh = Bm * np.asarray(0.5, dtype=ml_dtypes.bfloat16)  # exact halving

    nbc, eh, mainw = plan["nbc"], plan["eh"], plan["mainw"]
    gangs = plan["gangs"]
    awidth = P * len(gangs)
    in_maps = []
    for k in range(NCORE):
        a_rows = np.zeros((P, awidth), dtype=ml_dtypes.bfloat16)
        for g, sz in enumerate(gangs):
            off = sum(gangs[:g])
            for q in range(sz):
                blk = nbc * k + off + q
                a_rows[32 * q:32 * q + 4, P * g:P * g + P] = \
                    A[:, P * blk:P * blk + P]

        b_colsk = np.empty((4, plan["bcols"]), dtype=ml_dtypes.bfloat16)
        cols = (P * (nbc * k + 1) + np.arange(mainw)) % npad
        b_colsk[:, 0:mainw] = Bm[:, cols]
        for a in range(nbc):
            c0 = (P * (nbc * k + a) + np.arange(P)) % npad
            ch = (P * (nbc * k + a + eh) + np.arange(P)) % npad
            base = mainw + 256 * a
            b_colsk[:, base:base + P] = Bh[:, c0]
            b_colsk[:, base + P:base + 256] = Bh[:, ch]
        in_maps.append(
            {"a_rows": a_rows, "b_cols": np.ascontiguousarray(b_colsk)}
        )
    return in_maps, A, Bm, n1, plan


def kernel(pred, gt, gt_fracTime, gt_ifMOF):
    from concourse import bass_utils

    ift = int(np.asarray(gt_fracTime))
    imf = int(np.asarray(gt_ifMOF))

    in_maps, A, Bm, n1, plan = _prepare(pred, gt, ift, imf)
    nc = _get_nc(plan)
    res = bass_utils.run_bass_kernel_spmd(nc, in_maps,
                                          core_ids=list(range(NCORE)))

    # T = sum_{i<j} |M| (all device accumulator columns are weight 1)
    T = 0.0
    for r in res.results:
        T += r["acc"].astype(np.float64).sum()

    # host closed form in fp64 over the same bf16 values the device used:
    # sum_{i<j} M = (sum_{i,j} M - sum_diag M) / 2
    A64 = A.astype(np.float64)
    B64 = Bm.astype(np.float64)
    S_all = (A64.sum(axis=1) * B64.sum(axis=1)).sum()
    D_diag = (A64 * B64).sum()
    S_half = (S_all - D_diag) / 2.0

    n_pairs = (float(n1) * float(n1) - float(n1)) / 2.0

    loss = 0.5 * (S_half + T) / 100.0 / n_pairs
    return np.asarray(np.float32(loss))


# revision 26
# speedup vs baseline: 1.5459x; 1.0098x over previous
"""Trainium2 Bass kernel for the pairwise concordance-index loss.

reference:
    loss = sum_{i<j, f_i=f_j=1} relu((p_i-p_j)(t_i-t_j)) / 100 / n_pairs

Math:
  Only flagged (f=1) entries contribute, so the host first COMPACTS the
  arrays to the n1 flagged entries (padded with zeros to NB*128), which
  shrinks the pairwise matrix from B^2 to ~(0.7B)^2.
  M[i,j] = (p_i-p_j)(t_i-t_j) = A^T B, rank 4:
      A = [u, 1, p, t],  B = [1, u, -t, -p],  u = p*t   (zeros in padding)
  sum relu(M) = 0.5*(sum M + sum |M|); sum M has an O(n) closed form done
  on the host in fp64; sum |M| is the O(n^2) part done on device.

Device decomposition (8 cores, identical program, data-sharded):
  NB row-blocks of 128 rows; core k owns NBC=NB/8 blocks as a quad gang
  (4 blocks) + duo gang (NBC-4). Each block processes cyclic column
  offsets e=0..NB/2 (cols 128(a+e) mod NB*128): e=1..NB/2-1 at weight 1;
  e=0 / e=NB/2 at weight 0.5 via 0.5-pre-scaled slab appendices
  (host-side), so all device sums have uniform weight.

Device structure (raw Bass, hand-rolled semaphores — no TileContext, so
the multi-microsecond scheduler prologue/epilogue is avoided):
  PE: per 2-bank PSUM tile, 2-4 K=4 bf16 matmuls packed into disjoint
  32-row PE groups via tile_position (concurrent matmuls always target
  distinct PSUM banks). Each tile is consumed by ONE fused abs-row-sum
  job on the DVE (tensor_reduce(apply_absolute_value)) or the ScalarE
  (activation(Abs, accum_out)), alternating in fill order so both
  engines stream without head-of-line blocking. PSUM is an 8-bank ring
  of four 2-bank buffers; the PE waits on the consumer semaphore before
  reusing a buffer. Input DMAs are split across the sync and scalar
  HWDGE queues, chunked so the first tiles can start early.
"""

import numpy as np

B = 8192
P = 128
NCORE = 8
CUT = 2560

_cache = {}


def _plan(n1):
    """Compile-time plan derived from the flagged count."""
    nb = max(1, -(-n1 // P))        # 128-row blocks needed
    nb = -(-nb // NCORE) * NCORE    # multiple of 8 (even)
    nbc = nb // NCORE               # blocks per core
    eh = nb // 2                    # antipodal offset (weight 1/2)
    mainw = P * (nbc - 1 + eh - 1)  # shared slab for e=1..eh-1
    tailw = nbc * 256               # per-block [e0/2 | e_h/2] appendix
    gangs = [4] * (nbc // 4) + ([nbc % 4] if nbc % 4 else [])
    w = P * (eh - 1)                # main cols per block
    nfull, rem = w // 512, w % 512
    return dict(nb=nb, nbc=nbc, eh=eh, mainw=mainw, tailw=tailw,
                bcols=mainw + tailw, gangs=gangs, nfull=nfull, rem=rem)


def _slots(plan):
    """Interleaved (gang, kind, s) slot list; duo first for early start."""
    gangs = plan["gangs"]
    per_gang = []
    for g, sz in enumerate(gangs):
        sl = [(g, "main", s) for s in range(plan["nfull"])]
        if plan["rem"]:
            sl.append((g, "rem", plan["nfull"]))
        sl.append((g, "tail", 0))
        per_gang.append(sl)
    order = []
    ng = len(per_gang)
    nmax = max(len(s) for s in per_gang)
    for i in range(nmax):
        for g in reversed(range(ng)):  # duo (last gang) first
            if i < len(per_gang[g]):
                order.append(per_gang[g][i])
    return order


def _layout(plan):
    """Ordered tile descriptors: matmul lists, reduce specs, DMA gates."""
    gangs, rem, mainw = plan["gangs"], plan["rem"], plan["mainw"]
    tiles = []
    for g, kind, s in _slots(plan):
        sz = gangs[g]
        off = sum(gangs[:g])
        ntiles = 2 if sz > 2 else 1
        per_tile = [[] for _ in range(ntiles)]
        if kind == "tail":
            # two sequential waves of N=128 so concurrent matmuls (distinct
            # PE row-groups) never share a PSUM bank
            for wave in range(2):
                for q in range(sz):
                    coff = mainw + 256 * (off + q) + 128 * wave
                    ti, bank = (q // 2, q % 2) if ntiles == 2 else (0, q)
                    per_tile[ti].append((q, coff, 128, bank, 128 * wave))
            width = 256
        else:
            width = 512 if kind == "main" else rem
            for q in range(sz):
                coff = P * (off + q) + 512 * s
                ti, bank = (q // 2, q % 2) if ntiles == 2 else (0, q)
                per_tile[ti].append((q, coff, width, bank, 0))
        for ti in range(ntiles):
            mms = per_tile[ti]
            nbank = len(set(b for (_, _, _, b, _) in mms))
            # input chunks this tile's matmuls read (DMA completions are
            # unordered across HWDGE queues, so each chunk gets its own sem)
            chunks = {"a"}
            for (q, coff, n, _b, _c) in mms:
                if coff < CUT:
                    chunks.add(f"b{q}h1")
                if coff + n > CUT:
                    chunks.add(f"b{q}h2")
            tiles.append(dict(acol=P * g, mms=mms, nbank=nbank, width=width,
                              fd=nbank * width, chunks=chunks))
    # engine assignment: alternate DVE/ACT in fill order (keeps both engines
    # streaming); swap one pair near the tail to balance total element count
    n = len(tiles)
    assign = ["V" if i % 2 == 0 else "A" for i in range(n)]
    if n >= 16:
        assign[14], assign[15] = assign[15], assign[14]
    for t, a in zip(tiles, assign):
        t["eng"] = a
    return tiles


def _build(plan):
    """Build + compile the raw Bass module (once per plan)."""
    import concourse.bacc as bacc
    import concourse.mybir as mybir

    f32 = mybir.dt.float32
    bf16 = mybir.dt.bfloat16
    nc = bacc.Bacc("TRN2", target_bir_lowering=False, debug=False,
                   num_devices=NCORE)

    gangs = plan["gangs"]
    bcols = plan["bcols"]
    awidth = P * len(gangs)
    tiles = _layout(plan)
    njobs = len(tiles)
    nV = sum(1 for t in tiles if t["eng"] == "V")

    a_dram = nc.dram_tensor("a_rows", [P, awidth], bf16, kind="ExternalInput")
    b_dram = nc.dram_tensor("b_cols", [4, bcols], bf16, kind="ExternalInput")
    acc_dram = nc.dram_tensor("acc", [P, njobs], f32, kind="ExternalOutput")

    a_sb = nc.alloc_sbuf_tensor("a_sb", [P, awidth], bf16)
    b_sb = nc.alloc_sbuf_tensor("b_sb", [P, bcols], bf16)
    acc_sb = nc.alloc_sbuf_tensor("acc_sb", [P, njobs], f32)
    ps = nc.alloc_psum_tensor("ps", [P, 8, 512], f32)

    sem_mm = nc.alloc_semaphore("sem_mm")    # tiles filled by PE
    sem_v = nc.alloc_semaphore("sem_v")      # DVE jobs done
    sem_a = nc.alloc_semaphore("sem_a")      # ACT jobs done
    sem_out = nc.alloc_semaphore("sem_out")  # output DMA done
    # one sem per input chunk: HWDGE completions are unordered across queues
    chunk_order = ["a", "b0h1", "b1h1", "b2h1", "b3h1",
                   "b0h2", "b1h2", "b2h2", "b3h2"]
    sem_chunk = {c: nc.alloc_semaphore(f"sem_in_{c}") for c in chunk_order}
    sems = [sem_mm, sem_v, sem_a, sem_out] + list(sem_chunk.values())
    nums = sorted(s.num for s in sems)
    assert nums[-1] - nums[0] == len(nums) - 1, nums
    semrange = range(nums[0], nums[-1] + 1)

    # per-tile bookkeeping for sync
    jobidx = {}
    counts = {"V": 0, "A": 0}
    for i, t in enumerate(tiles):
        counts[t["eng"]] += 1
        jobidx[i] = counts[t["eng"]]  # 1-based within its engine

    with nc.Block("k") as blk:

        @blk.sync
        def _(eng):
            # everything the first tiles need goes on this (faster) queue,
            # most-critical first
            for q in (0, 1):
                eng.dma_start(b_sb.ap()[32 * q:32 * q + 4, 0:CUT],
                              b_dram.ap()[:, 0:CUT]).then_inc(
                                  sem_chunk[f"b{q}h1"], 16)
            eng.dma_start(a_sb.ap()[:, :], a_dram.ap()[:, :]).then_inc(
                sem_chunk["a"], 16)
            eng.dma_start(b_sb.ap()[32 * 2:32 * 2 + 4, 0:CUT],
                          b_dram.ap()[:, 0:CUT]).then_inc(
                              sem_chunk["b2h1"], 16)
            for q in (0, 2):
                eng.dma_start(b_sb.ap()[32 * q:32 * q + 4, CUT:bcols],
                              b_dram.ap()[:, CUT:bcols]).then_inc(
                                  sem_chunk[f"b{q}h2"], 16)

        @blk.tensor
        def _(eng):
            waited = set()
            for i, t in enumerate(tiles):
                for c in chunk_order:
                    if c in t["chunks"] and c not in waited:
                        waited.add(c)
                        eng.wait_ge(sem_chunk[c], 16)
                if i >= 4:
                    p = i - 4  # previous occupant of this 2-bank buffer
                    eng.wait_ge(sem_v if tiles[p]["eng"] == "V" else sem_a,
                                jobidx[p])
                buf = i % 4
                last = len(t["mms"]) - 1
                for j, (q, coff, n, bank, c0) in enumerate(t["mms"]):
                    ins = nc.tensor.matmul(
                        ps.ap()[:, 2 * buf + bank, c0:c0 + n],
                        a_sb.ap()[32 * q:32 * q + 4, t["acol"]:t["acol"] + P],
                        b_sb.ap()[32 * q:32 * q + 4, coff:coff + n],
                        start=True,
                        stop=True,
                        tile_position=(32 * q, 0),
                    )
                    if j == last:
                        ins.then_inc(sem_mm, 1)

        @blk.vector
        def _(eng):
            for i, t in enumerate(tiles):
                if t["eng"] != "V":
                    continue
                buf = i % 4
                eng.wait_ge(sem_mm, i + 1)
                eng.tensor_reduce(
                    acc_sb.ap()[:, i:i + 1],
                    ps.ap()[:, 2 * buf:2 * buf + t["nbank"], 0:t["width"]],
                    axis=mybir.AxisListType.XY, op=mybir.AluOpType.add,
                    apply_absolute_value=True,
                ).then_inc(sem_v, 1)

        @blk.scalar
        def _(eng):
            eng.dma_start(b_sb.ap()[32 * 3:32 * 3 + 4, 0:CUT],
                          b_dram.ap()[:, 0:CUT]).then_inc(
                              sem_chunk["b3h1"], 16)
            for q in (1, 3):
                eng.dma_start(b_sb.ap()[32 * q:32 * q + 4, CUT:bcols],
                              b_dram.ap()[:, CUT:bcols]).then_inc(
                                  sem_chunk[f"b{q}h2"], 16)
            for i, t in enumerate(tiles):
                if t["eng"] != "A":
                    continue
                buf = i % 4
                eng.wait_ge(sem_mm, i + 1)
                red = ps.ap()[:, 2 * buf:2 * buf + t["nbank"], 0:t["width"]]
                eng.activation(
                    red, red,
                    mybir.ActivationFunctionType.Abs,
                    accum_out=acc_sb.ap()[:, i:i + 1],
                ).then_inc(sem_a, 1)
            # explicit edges: ACT's own accumulator writes + DVE columns
            eng.wait_ge(sem_a, njobs - nV)
            eng.wait_ge(sem_v, nV)
            eng.dma_start(acc_dram.ap()[:, :], acc_sb.ap()[:, :]).then_inc(
                sem_out, 16)
            eng.wait_ge(sem_out, 16)

    # reset semaphores so repeated executions of this NEFF start clean;
    # the block exit drained every engine, the barrier orders the clear
    # after all of them (each sem's final value was waited in-program).
    import os
    if not os.environ.get("SKIP_SEM_CLEAR"):
        nc.all_engine_barrier()
        nc.gpsimd.dma_reset(semrange)
        nc.gpsimd.sem_clear(semrange)

    nc.compile()
    return nc


def _get_nc(plan):
    key = ("nc", plan["nb"])
    if key not in _cache:
        _cache[key] = _build(plan)
    return _cache[key]


def _prepare(pred, gt, ift, imf):
    """Compact + pad + build per-core input maps."""
    import ml_dtypes

    p_full = np.asarray(pred).astype(np.float32)
    gt = np.asarray(gt).astype(np.float32)
    t_full = gt[:, ift]
    f_full = gt[:, imf] == 1
    idx = np.flatnonzero(f_full)
    n1 = len(idx)

    plan = _plan(n1)
    npad = plan["nb"] * P
    p = np.zeros(npad, np.float32)
    t = np.zeros(npad, np.float32)
    w = np.zeros(npad, np.float32)
    p[:n1] = p_full[idx]
    t[:n1] = t_full[idx]
    w[:n1] = 1.0
    u = p * t

    # compaction makes the flags trivial: real entries are all flagged,
    # padded entries are exactly zero in every factor.
    A = np.ascontiguousarray(
        np.stack([u, w, p, t]).astype(ml_dtypes.bfloat16)
    )
    Bm = np.ascontiguousarray(
        np.stack([w, u, -t, -p]).astype(ml_dtypes.bfloat16)
    )
    Bh = Bm * np.asarray(0.5, dtype=ml_dtypes.bfloat16)  # exact halving

    nbc, eh, mainw = plan["nbc"], plan["eh"], plan["mainw"]
    gangs = plan["gangs"]
    awidth = P * len(gangs)
    in_maps = []
    for k in range(NCORE):
        a_rows = np.zeros((P, awidth), dtype=ml_dtypes.bfloat16)
        for g, sz in enumerate(gangs):
            off = sum(gangs[:g])
            for q in range(sz):
                blk = nbc * k + off + q
                a_rows[32 * q:32 * q + 4, P * g:P * g + P] = \
                    A[:, P * blk:P * blk + P]

        b_colsk = np.empty((4, plan["bcols"]), dtype=ml_dtypes.bfloat16)
        cols = (P * (nbc * k + 1) + np.arange(mainw)) % npad
        b_colsk[:, 0:mainw] = Bm[:, cols]
        for a in range(nbc):
            c0 = (P * (nbc * k + a) + np.arange(P)) % npad
            ch = (P * (nbc * k + a + eh) + np.arange(P)) % npad
            base = mainw + 256 * a
            b_colsk[:, base:base + P] = Bh[:, c0]
            b_colsk[:, base + P:base + 256] = Bh[:, ch]
        in_maps.append(
            {"a_rows": a_rows, "b_cols": np.ascontiguousarray(b_colsk)}
        )
    return in_maps, A, Bm, n1, plan


def kernel(pred, gt, gt_fracTime, gt_ifMOF):
    from concourse import bass_utils

    ift = int(np.asarray(gt_fracTime))
    imf = int(np.asarray(gt_ifMOF))

    in_maps, A, Bm, n1, plan = _prepare(pred, gt, ift, imf)
    nc = _get_nc(plan)
    res = bass_utils.run_bass_kernel_spmd(nc, in_maps,
                                          core_ids=list(range(NCORE)))

    # T = sum_{i<j} |M| (all device accumulator columns are weight 1)
    T = 0.0
    for r in res.results:
        T += r["acc"].astype(np.float64).sum()

    # host closed form in fp64 over the same bf16 values the device used:
    # sum_{i<j} M = (sum_{i,j} M - sum_diag M) / 2
    A64 = A.astype(np.float64)
    B64 = Bm.astype(np.float64)
    S_all = (A64.sum(axis=1) * B64.sum(axis=1)).sum()
    D_diag = (A64 * B64).sum()
    S_half = (S_all - D_diag) / 2.0

    n_pairs = (float(n1) * float(n1) - float(n1)) / 2.0

    loss = 0.5 * (S_half + T) / 100.0 / n_pairs
    return np.asarray(np.float32(loss))
